# revision 1
# baseline (speedup 1.0000x reference)
"""Trainium2 Bass kernel for nn_NonLocalNd_bn_cbam (non-local attention + BN
whitening + global-context branch), data-parallel over batch on 8 NeuronCores.

Hardcoded problem shape: x [8, 256, 64, 64], P=128 projections, maxpool2x2 for
k/v (Nk=1024), Nq=4096.  Each core handles one batch element; the only
cross-core coupling is the BatchNorm whitening statistics of q and k, handled
by a tiny [128, 4] AllReduce.

Math restructuring (validated against the jax reference: ~1e-4 rel err with
bf16 matmul operands, fp32 accumulation and fp32 residual):
  - spatial whitening (subtract channel-mean) is linear -> folded into w_q/w_k
    on the host:  W~ = (I - 11^T/128) W,  b~ = (I - 11^T/128) b
  - b_mask drops out of softmax entirely.
  - attention computed in transposed layout simT[m, n] = sum_p k[p,m] q[p,n];
    softmax denominator via ones-vector matmuls; exp without max-subtraction
    (|sim/scale| <= ~8 for whitened q,k); division deferred until after the
    attn@v matmul.
  - gamma folded into the ones-vector (colsum' = colsum/gamma), so
    out_sim = av * recip(colsum') directly includes gamma; gc branch unscaled.

Schedule: phase 1 computes maxpool + q/k projections + local BN moments and
fires the AllReduce as early as possible; the v-transpose / mask / gc work and
the fp32 residual load run inside the collective's latency window; attention
blocks fuse the output projection of the previous block into their PE stream.
"""

import math

import ml_dtypes
import numpy as np

import concourse.bass as bass
import concourse.mybir as mybir
import concourse.tile as tile
from concourse import bacc
from concourse.bass_isa import ReduceOp
from concourse.bass_utils import run_bass_kernel_spmd

F32 = mybir.dt.float32
BF16 = mybir.dt.bfloat16
AF = mybir.ActivationFunctionType
OP = mybir.AluOpType
AX = mybir.AxisListType

B, CIN, H, W = 8, 256, 64, 64
P = 128
NQ = H * W                # 4096
NK = (H // 2) * (W // 2)  # 1024
N_CORES = 8
EPS = 1e-5
INV_SCALE = 1.0 / math.sqrt(P)   # temperature 1.0

LAST_RESULTS = None  # test harness reads exec_time from here


def _maybe_shim_trace_hooks():
    """If BASS_TRACE is set in the environment, bass_utils imports
    antenv.axon_hooks, which this container image lacks.  Recreate it (and
    stub the artifact upload) so tracing degrades gracefully instead of
    crashing; a failure here is harmless for the non-traced path."""
    import os
    import sys
    import types

    if not os.environ.get("BASS_TRACE"):
        return
    try:
        import antenv.axon_hooks  # noqa: F401
        return
    except ImportError:
        pass
    try:
        import antenv
        from trn_agent_boot.trn_boot import _ntff_profile_via_ctypes

        hook = _ntff_profile_via_ctypes("/opt/axon/libaxon_pjrt.so")
        m = types.ModuleType("antenv.axon_hooks")
        m.get_axon_ntff_profile_hook = lambda: hook
        m.set_axon_ntff_profile_hook = lambda h: None
        sys.modules["antenv.axon_hooks"] = m
        antenv.axon_hooks = m
        from concourse import bass_utils as _bu

        _bu.upload_artifacts = lambda tmpdir: tmpdir
    except Exception:
        os.environ["BASS_NEVER_TRACE"] = "1"


def _build_bass(inv_gamma: float):
    nc = bacc.Bacc("TRN2", target_bir_lowering=False)

    # ---- per-core I/O ----------------------------------------------------
    x_d = nc.dram_tensor("x", [CIN, NQ], F32, kind="ExternalInput")
    xb_d = nc.dram_tensor("xb", [CIN, NQ], BF16, kind="ExternalInput")
    # packed bf16 weights: [2, 128, 385] = (wqT | wkT | wvT | wmT) chunked
    wcat_d = nc.dram_tensor("wcat", [2, 128, 385], BF16, kind="ExternalInput")
    bcat_d = nc.dram_tensor("bcat", [P, 2], F32, kind="ExternalInput")  # bq|bk
    bv_d = nc.dram_tensor("bv", [1, P], F32, kind="ExternalInput")
    woutT_d = nc.dram_tensor("woutT", [P, CIN], BF16, kind="ExternalInput")
    out_d = nc.dram_tensor("out", [CIN, NQ], F32, kind="ExternalOutput")

    groups = [list(range(N_CORES))]

    with tile.TileContext(nc) as tc:
        with (
            tc.tile_pool(name="consts", bufs=1) as consts,
            tc.tile_pool(name="bigs", bufs=1) as bigs,
            tc.tile_pool(name="mp", bufs=4) as mp,
            tc.tile_pool(name="small", bufs=1) as small,
            tc.tile_pool(name="dram", bufs=1, space="DRAM") as dramp,
        ):
            cc_in_d = dramp.tile([P, 4], F32, tag="cc_in")
            cc_out_d = dramp.tile([P, 4], F32, tag="cc_out", addr_space="Shared")

            # ---- input loads (xb first: everything hangs off it) --------
            xb_sb = [bigs.tile([128, NQ], BF16, name=f"xb{ct}", tag=f"xb{ct}") for ct in range(2)]
            for ct in range(2):
                for half in range(2):
                    nc.sync.dma_start(
                        out=xb_sb[ct][:, half * 2048:(half + 1) * 2048],
                        in_=xb_d[ct * 128:(ct + 1) * 128, half * 2048:(half + 1) * 2048],
                    )
            wcat_t = consts.tile([128, 2, 385], BF16, tag="wcat")
            for cc in range(2):
                nc.sync.dma_start(out=wcat_t[:, cc, :], in_=wcat_d[cc, :, :])
            bcat_t = consts.tile([128, 2], F32, tag="bcat")
            nc.sync.dma_start(out=bcat_t, in_=bcat_d[:, :])
            bv_row = consts.tile([1, 128], F32, tag="bvrow")
            nc.sync.dma_start(out=bv_row, in_=bv_d[:, :])
            wout_t = consts.tile([128, CIN], BF16, tag="wout")
            nc.sync.dma_start(out=wout_t, in_=woutT_d[:, :])

            def wq(cc):
                return wcat_t[:, cc, 0:128]

            def wk(cc):
                return wcat_t[:, cc, 128:256]

            def wv(cc):
                return wcat_t[:, cc, 256:384]

            def wm(cc):
                return wcat_t[:, cc, 384:385]

            ones_t = consts.tile([128, 1], BF16, tag="ones")
            nc.vector.memset(ones_t, inv_gamma)
            eps_t = small.tile([128, 1], F32, tag="eps")
            nc.vector.memset(eps_t, EPS)

            # ---- maxpool (split between DVE and GpSimd) ------------------
            xp_sb = [bigs.tile([128, NK], BF16, name=f"xp{ct}", tag=f"xp{ct}") for ct in range(2)]
            for q in range(4):
                for ct in range(2):
                    xv = xb_sb[ct][:, q * 1024:(q + 1) * 1024].rearrange(
                        "p (i a j b) -> p i a j b", i=8, a=2, j=32, b=2
                    )
                    t1 = mp.tile([128, 8, 32], BF16, name=f"t1_{q}_{ct}", tag="mp1")
                    t2 = mp.tile([128, 8, 32], BF16, name=f"t2_{q}_{ct}", tag="mp2")
                    xo = xp_sb[ct][:, q * 256:(q + 1) * 256].rearrange(
                        "p (i j) -> p i j", i=8
                    )
                    nc.vector.tensor_max(t1, xv[:, :, 0, :, 0], xv[:, :, 0, :, 1])
                    nc.vector.tensor_max(t2, xv[:, :, 1, :, 0], xv[:, :, 1, :, 1])
                    nc.vector.tensor_max(xo, t1, t2)

            qc = bigs.tile([128, NQ], F32, tag="qc")
            kc = bigs.tile([128, NK], F32, tag="kc")
            stats_q = small.tile([128, 8, 6], F32, tag="stats_q")
            stats_k = small.tile([128, 2, 6], F32, tag="stats_k")

            with (
                tc.tile_pool(name="ps1", bufs=2, space="PSUM") as ps_q,
                tc.tile_pool(name="ps1k", bufs=1, space="PSUM") as ps_k,
                tc.tile_pool(name="ps1v", bufs=2, space="PSUM") as ps_v,
                tc.tile_pool(name="ps1m", bufs=1, space="PSUM") as ps_m,
                tc.tile_pool(name="ps1g", bufs=1, space="PSUM") as ps_g,
            ):
                # ---- q projection + per-chunk stats ----------------------
                for j in range(8):
                    qp = ps_q.tile([128, 512], F32, name=f"qp{j}", tag="qp")
                    for cc in range(2):
                        nc.tensor.matmul(
                            qp,
                            wq(cc),
                            xb_sb[cc][:, j * 512:(j + 1) * 512],
                            start=(cc == 0),
                            stop=(cc == 1),
                        )
                    nc.scalar.activation(
                        qc[:, j * 512:(j + 1) * 512], qp, AF.Identity,
                        bias=bcat_t[:, 0:1],
                    )
                    nc.vector.bn_stats(stats_q[:, j, :], qc[:, j * 512:(j + 1) * 512])

                # ---- k projection + stats --------------------------------
                kp = ps_k.tile([128, NK], F32, tag="kp")
                for hh in range(2):
                    for cc in range(2):
                        nc.tensor.matmul(
                            kp[:, hh * 512:(hh + 1) * 512],
                            wk(cc),
                            xp_sb[cc][:, hh * 512:(hh + 1) * 512],
                            start=(cc == 0),
                            stop=(cc == 1),
                        )
                nc.scalar.activation(kc, kp, AF.Identity, bias=bcat_t[:, 1:2])
                for hh in range(2):
                    nc.vector.bn_stats(stats_k[:, hh, :], kc[:, hh * 512:(hh + 1) * 512])

                # ---- global BN stats AllReduce (fire ASAP) ---------------
                mv_q = small.tile([128, 2], F32, tag="mv_q")
                mv_k = small.tile([128, 2], F32, tag="mv_k")
                nc.vector.bn_aggr(mv_q, stats_q)
                nc.vector.bn_aggr(mv_k, stats_k)
                cc_sb = small.tile([128, 4], F32, tag="cc_sb")
                tq = small.tile([128, 1], F32, tag="tq")
                nc.vector.tensor_scalar(
                    out=cc_sb[:, 0:1], in0=mv_q[:, 0:1], scalar1=float(NQ),
                    scalar2=None, op0=OP.mult,
                )
                nc.vector.tensor_mul(tq, mv_q[:, 0:1], mv_q[:, 0:1])
                nc.vector.tensor_add(tq, tq, mv_q[:, 1:2])
                nc.vector.tensor_scalar(
                    out=cc_sb[:, 1:2], in0=tq, scalar1=float(NQ),
                    scalar2=None, op0=OP.mult,
                )
                tk = small.tile([128, 1], F32, tag="tk")
                nc.vector.tensor_scalar(
                    out=cc_sb[:, 2:3], in0=mv_k[:, 0:1], scalar1=float(NK),
                    scalar2=None, op0=OP.mult,
                )
                nc.vector.tensor_mul(tk, mv_k[:, 0:1], mv_k[:, 0:1])
                nc.vector.tensor_add(tk, tk, mv_k[:, 1:2])
                nc.vector.tensor_scalar(
                    out=cc_sb[:, 3:4], in0=tk, scalar1=float(NK),
                    scalar2=None, op0=OP.mult,
                )
                nc.sync.dma_start(out=cc_in_d[:, :], in_=cc_sb)
                nc.gpsimd.collective_compute(
                    "AllReduce", OP.add, replica_groups=groups,
                    ins=[cc_in_d.opt()], outs=[cc_out_d.opt()],
                )
                g_sb = small.tile([128, 4], F32, tag="g_sb")
                nc.sync.dma_start(out=g_sb, in_=cc_out_d[:, :])

                # ---- collective-window work: vT, mask, gc ----------------
                bv_bc = consts.tile([128, 128], F32, tag="bvbc")
                nc.gpsimd.partition_broadcast(bv_bc, bv_row, 128)
                vT = [bigs.tile([128, 128], BF16, name=f"vt{mc}", tag=f"vt{mc}") for mc in range(8)]
                for mc in range(8):
                    vp = ps_v.tile([128, 128], F32, name=f"vp{mc}", tag="vp")
                    for cc in range(2):
                        nc.tensor.matmul(
                            vp,
                            xp_sb[cc][:, mc * 128:(mc + 1) * 128],
                            wv(cc),
                            start=(cc == 0),
                            stop=(cc == 1),
                        )
                    nc.vector.scalar_tensor_tensor(
                        out=vT[mc], in0=vp, scalar=1.0, in1=bv_bc,
                        op0=OP.mult, op1=OP.add,
                    )

                mt = ps_m.tile([128, 8], F32, tag="mt")
                for mc in range(8):
                    for cc in range(2):
                        nc.tensor.matmul(
                            mt[:, mc:mc + 1],
                            xp_sb[cc][:, mc * 128:(mc + 1) * 128],
                            wm(cc),
                            start=(cc == 0),
                            stop=(cc == 1),
                        )
                em = small.tile([128, 8], BF16, tag="em")
                nc.scalar.activation(em, mt, AF.Exp)
                s1 = small.tile([128, 1], F32, tag="s1")
                nc.vector.reduce_sum(s1, em, axis=AX.X)
                s_bc = small.tile([128, 1], F32, tag="s_bc")
                nc.gpsimd.partition_all_reduce(s_bc, s1, 128, ReduceOp.add)
                r_s = small.tile([128, 1], F32, tag="r_s")
                nc.vector.reciprocal_approx_fast(out=r_s, in_=s_bc)
                gcp = ps_g.tile([128, 1], F32, tag="gcp")
                for mc in range(8):
                    nc.tensor.matmul(
                        gcp, vT[mc], em[:, mc:mc + 1],
                        start=(mc == 0), stop=(mc == 7),
                    )
                gc_t = small.tile([128, 1], F32, tag="gc")
                nc.vector.tensor_scalar(
                    out=gc_t, in0=gcp, scalar1=r_s, scalar2=None, op0=OP.mult
                )

                # ---- normalization params: rstd = exp(-0.5*ln(var+eps)) --
                qn = bigs.tile([128, NQ], BF16, tag="qn")
                kn = bigs.tile([128, NK], BF16, tag="kn")
                for (sl, inv_n, src, dst) in (
                    (0, 1.0 / (B * NQ), qc, qn),
                    (2, 1.0 / (B * NK), kc, kn),
                ):
                    gm = small.tile([128, 1], F32, name=f"gm{sl}", tag=f"gm{sl}")
                    e2 = small.tile([128, 1], F32, name=f"e2{sl}", tag=f"e2{sl}")
                    var = small.tile([128, 1], F32, name=f"var{sl}", tag=f"var{sl}")
                    nc.vector.tensor_scalar(
                        out=gm, in0=g_sb[:, sl:sl + 1], scalar1=inv_n,
                        scalar2=None, op0=OP.mult,
                    )
                    nc.vector.tensor_scalar(
                        out=e2, in0=g_sb[:, sl + 1:sl + 2], scalar1=inv_n,
                        scalar2=None, op0=OP.mult,
                    )
                    nc.vector.tensor_mul(var, gm, gm)
                    nc.vector.tensor_sub(var, e2, var)
                    lnv = small.tile([128, 1], F32, name=f"lnv{sl}", tag=f"lnv{sl}")
                    nc.scalar.activation(lnv, var, AF.Ln, bias=eps_t)
                    rstd = small.tile([128, 1], F32, name=f"rstd{sl}", tag=f"rstd{sl}")
                    nc.scalar.activation(rstd, lnv, AF.Exp, scale=-0.5)
                    nbias = small.tile([128, 1], F32, name=f"nb{sl}", tag=f"nb{sl}")
                    nc.vector.tensor_scalar(
                        out=nbias, in0=gm, scalar1=rstd, scalar2=-1.0,
                        op0=OP.mult, op1=OP.mult,
                    )
                    nc.vector.tensor_scalar(
                        out=dst, in0=src, scalar1=rstd, scalar2=nbias,
                        op0=OP.mult, op1=OP.add,
                    )

            # ---- residual input loads during attention -------------------
            x_sb = [bigs.tile([128, NQ], F32, name=f"x{ct}", tag=f"x{ct}") for ct in range(2)]
            for j in range(4):
                for ct in range(2):
                    nc.sync.dma_start(
                        out=x_sb[ct][:, j * 1024:(j + 1) * 1024],
                        in_=x_d[ct * 128:(ct + 1) * 128, j * 1024:(j + 1) * 1024],
                    )

            # ---- phase 2: attention + fused output projection ------------
            outsim = bigs.tile([128, NQ], BF16, tag="outsim")
            with (
                tc.tile_pool(name="ps_sim", bufs=2, space="PSUM") as ps_sim,
                tc.tile_pool(name="ps_cs", bufs=2, space="PSUM") as ps_cs,
                tc.tile_pool(name="ps_av", bufs=1, space="PSUM") as ps_av,
                tc.tile_pool(name="epool", bufs=10) as epool,
                tc.tile_pool(name="rows", bufs=2) as rows,
                tc.tile_pool(name="rbcp", bufs=2) as rbcp,
                tc.tile_pool(name="outp", bufs=3) as outp,
            ):
                def flush_out(j):
                    # out[c, nb] = w_out @ outsim[:, nb] + x[c, nb] -> DRAM
                    for ct in range(2):
                        op = ps_sim.tile([128, 1024], F32, name=f"op{j}_{ct}", tag="sim")
                        for hh in range(2):
                            nc.tensor.matmul(
                                op[:, hh * 512:(hh + 1) * 512],
                                wout_t[:, ct * 128:(ct + 1) * 128],
                                outsim[:, j * 1024 + hh * 512:j * 1024 + (hh + 1) * 512],
                                start=True, stop=True,
                            )
                        ot = outp.tile([128, 1024], F32, name=f"ot{j}_{ct}", tag="ot")
                        nc.vector.tensor_add(
                            ot, op, x_sb[ct][:, j * 1024:(j + 1) * 1024]
                        )
                        nc.sync.dma_start(
                            out=out_d[ct * 128:(ct + 1) * 128, j * 1024:(j + 1) * 1024],
                            in_=ot,
                        )

                for b in range(4):
                    nb = b * 1024
                    es = []
                    for mc in range(8):
                        sim = ps_sim.tile([128, 1024], F32, name=f"sim{b}_{mc}", tag="sim")
                        for hh in range(2):
                            nc.tensor.matmul(
                                sim[:, hh * 512:(hh + 1) * 512],
                                kn[:, mc * 128:(mc + 1) * 128],
                                qn[:, nb + hh * 512:nb + (hh + 1) * 512],
                                start=True, stop=True,
                            )
                        e_t = epool.tile([128, 1024], BF16, name=f"e{b}_{mc}", tag="e")
                        nc.scalar.activation(e_t, sim, AF.Exp, scale=INV_SCALE)
                        es.append(e_t)
                    if b >= 1:
                        flush_out(b - 1)
                    # colsum sweep (denominator / gamma)
                    cs0 = ps_cs.tile([1, 512], F32, name=f"cs0_{b}", tag="cs")
                    cs1 = ps_cs.tile([1, 512], F32, name=f"cs1_{b}", tag="cs")
                    for mc in range(8):
                        nc.tensor.matmul(
                            cs0, ones_t, es[mc][:, 0:512],
                            start=(mc == 0), stop=(mc == 7),
                        )
                        nc.tensor.matmul(
                            cs1, ones_t, es[mc][:, 512:1024],
                            start=(mc == 0), stop=(mc == 7),
                        )
                    # attn @ v sweep
                    av = ps_av.tile([128, 1024], F32, name=f"av{b}", tag="av")
                    for mc in range(8):
                        for hh in range(2):
                            nc.tensor.matmul(
                                av[:, hh * 512:(hh + 1) * 512],
                                vT[mc],
                                es[mc][:, hh * 512:(hh + 1) * 512],
                                start=(mc == 0), stop=(mc == 7),
                            )
                    # reciprocal row -> broadcast -> normalize (+gc)
                    csrow = rows.tile([1, 1024], F32, name=f"csr{b}", tag="csrow")
                    nc.vector.tensor_copy(csrow[:, 0:512], cs0)
                    nc.vector.tensor_copy(csrow[:, 512:1024], cs1)
                    rrow = rows.tile([1, 1024], F32, name=f"rr{b}", tag="rrow")
                    nc.vector.reciprocal_approx_fast(out=rrow, in_=csrow)
                    rbc = rbcp.tile([128, 1024], F32, name=f"rbc{b}", tag="rbc")
                    nc.gpsimd.partition_broadcast(rbc, rrow, 128)
                    nc.vector.tensor_mul(outsim[:, nb:nb + 1024], av, rbc)
                    nc.vector.tensor_scalar(
                        out=outsim[:, nb:nb + 1024], in0=outsim[:, nb:nb + 1024],
                        scalar1=gc_t, scalar2=None, op0=OP.add,
                    )
                flush_out(3)

    nc.compile()
    return nc


def kernel(x, w_q, b_q, w_k, b_k, w_v, b_v, w_out, w_mask, b_mask, gamma):
    global LAST_RESULTS
    x = np.ascontiguousarray(np.asarray(x, dtype=np.float32))
    gamma_f = float(np.asarray(gamma).reshape(-1)[0])
    inv_gamma = float(1.0 / gamma_f) if gamma_f != 0.0 else float("inf")

    # fold spatial whitening (subtract channel-mean over P) into q/k weights
    C = np.eye(P, dtype=np.float64) - 1.0 / P
    wq = (C @ np.asarray(w_q, dtype=np.float64)).astype(np.float32)
    bq = (C @ np.asarray(b_q, dtype=np.float64)).astype(np.float32)
    wk = (C @ np.asarray(w_k, dtype=np.float64)).astype(np.float32)
    bk = (C @ np.asarray(b_k, dtype=np.float64)).astype(np.float32)

    bf = ml_dtypes.bfloat16
    wcat = np.concatenate(
        [
            wq.T,
            wk.T,
            np.asarray(w_v, np.float32).T,
            np.asarray(w_mask, np.float32).T,
        ],
        axis=1,
    ).astype(bf)                                     # [256, 385]
    base = {
        "wcat": np.ascontiguousarray(wcat.reshape(2, 128, 385)),
        "bcat": np.ascontiguousarray(
            np.stack([bq, bk], axis=1).astype(np.float32)
        ),                                            # [128, 2]
        "bv": np.ascontiguousarray(np.asarray(b_v, np.float32).reshape(1, P)),
        "woutT": np.ascontiguousarray(np.asarray(w_out, np.float32).T.astype(bf)),
    }
    xf = x.reshape(B, CIN, NQ)
    xbf = xf.astype(bf)
    in_maps = [
        dict(base, x=np.ascontiguousarray(xf[c]), xb=np.ascontiguousarray(xbf[c]))
        for c in range(N_CORES)
    ]

    _maybe_shim_trace_hooks()
    nc = _build_bass(inv_gamma)
    res = run_bass_kernel_spmd(nc, in_maps, list(range(N_CORES)))
    LAST_RESULTS = res

    out = np.stack([res.results[c]["out"] for c in range(N_CORES)], axis=0)
    return out.reshape(B, CIN, H, W).astype(np.float32)

